# revision 17
# baseline (speedup 1.0000x reference)
"""Causal multi-head attention (B=4, L=1024, D=1024, H=16) on 8 TRN2 NeuronCores.

Sharding: core c = 2*b + g handles batch b (0..3) and head group g (0..1,
8 heads each).  Each core computes QKV projections for its heads, causal
attention (upper-triangle blocks skipped; the mask and rel-pos bias are
folded into a host-packed MULTIPLICATIVE table exp(bias + mask)), and a
PARTIAL output projection against its 512 rows of w_out.  The two cores of
a batch return partial [D, L] outputs that the host sums and transposes —
no on-device collectives.

Layouts are chosen so nothing is ever transposed on device:
 - qT/kT live as [head_dim(64) on partitions, tok]; the scores matmul
   emits scores^T [kpos, q] directly.
 - v lives as [tok on partitions, 64] with a ones column appended, so the
   ctx matmul ctxT[d, q] = sum_k v[k, d] p[k, q] also accumulates softmax
   denominators into ctx row 64 for free.
 - softmax skips max-subtraction (scores are O(6); exp is safe):
   p = exp(s) * expbias, denominators divide the small [64, L] ctx rows.
 - the denominator reciprocal row is broadcast across partitions with a
   K=1 matmul (ones[1,64]^T @ recip[1,L]) — partition-crossing data moves
   only ever happen on PE or DMA.
"""

import functools

import ml_dtypes
import numpy as np

B, L, D, H = 4, 1024, 1024, 16
HD = D // H  # 64
HPC = H // 2  # heads per core group = 8
P = 128
NT = L // P  # 8 token/query/key tiles
NEG = -1e30

BF16 = ml_dtypes.bfloat16

# packed bias geometry, (pair, j)-major: block (pair, j) holds the head
# pair's two [128, W_j] slabs side by side
_W = [L - P * j for j in range(NT)]
_OFF_J = [0] * NT
for _j in range(1, NT):
    _OFF_J[_j] = _OFF_J[_j - 1] + 2 * _W[_j - 1]
_PAIR_COLS = _OFF_J[-1] + 2 * _W[-1]  # 2 * 4608
_BIAS_COLS = (HPC // 2) * _PAIR_COLS


def _regions(j):
    """q-ranges of the causal suffix [128j, L), split at the 512 psum-bank
    boundary so each matmul output stays inside one bank."""
    a = P * j
    if a < 512:
        return [(a, 512), (512, 1024)]
    return [(a, 1024)]


@functools.lru_cache(maxsize=1)
def _build():
    import concourse.mybir as mybir
    import concourse.tile as tile
    from concourse import bacc

    f32 = mybir.dt.float32
    f32r = mybir.dt.float32r
    bf16 = mybir.dt.bfloat16
    Exp = mybir.ActivationFunctionType.Exp

    nc = bacc.Bacc(None, target_bir_lowering=False, debug=False)

    xt_d = nc.dram_tensor("xt", [D, L], bf16, kind="ExternalInput")
    # wqk packed host-side as 8 col-tile blocks of [D, 128] so each block is
    # one small DMA and the first matmuls start early
    wqk_d = nc.dram_tensor("wqk", [NT * D, P], bf16, kind="ExternalInput")
    wv_d = nc.dram_tensor("wv", [D, HPC * HD], bf16, kind="ExternalInput")
    wout_d = nc.dram_tensor("wout", [HPC * HD, D], bf16, kind="ExternalInput")
    bias_d = nc.dram_tensor("biasp", [P, _BIAS_COLS], bf16, kind="ExternalInput")
    outp_d = nc.dram_tensor("outp", [D, L], f32, kind="ExternalOutput")

    with tile.TileContext(nc) as tc:
        with (
            tc.tile_pool(name="persist", bufs=1) as keep,
            tc.tile_pool(name="bias", bufs=2) as bias_pool,
            tc.tile_pool(name="ptr", bufs=3) as ptr_pool,
            tc.tile_pool(name="pt", bufs=3) as pt_pool,
            tc.tile_pool(name="recip", bufs=2) as r_pool,
            tc.tile_pool(name="stg", bufs=2) as stg_pool,
            tc.tile_pool(name="osb", bufs=2) as out_pool,
            tc.tile_pool(name="psum", bufs=2, space="PSUM") as psum,
        ):
            xt_sb = keep.tile([P, NT, L], bf16, tag="xt")
            wqk_sb = keep.tile([P, NT, 2 * HPC * HD], bf16, tag="wqk")
            wv_sb = keep.tile([P, NT, HPC * HD], bf16, tag="wv")
            wout_sb = keep.tile([P, HPC * HD // P, D], bf16, tag="wout")
            qt_sb = keep.tile([P, HPC // 2, L], bf16, tag="qt")
            kt_sb = keep.tile([P, HPC // 2, L], bf16, tag="kt")
            vaug_sb = keep.tile([P, NT, HPC, HD + 1], bf16, tag="vaug")
            ctx_sb = keep.tile([P, HPC * HD // P, L], bf16, tag="ctx")
            ones_sb = keep.tile([65, HD], bf16, tag="ones")

            def dma_wqk(t):
                nc.sync.dma_start(
                    wqk_sb[:, :, P * t : P * (t + 1)],
                    wqk_d[t * D : (t + 1) * D, :].rearrange("(dt p) c -> p dt c", p=P),
                )

            def dma_xt(u):
                nc.sync.dma_start(
                    xt_sb[:, :, 512 * u : 512 * (u + 1)],
                    xt_d.rearrange("(t p) n -> p t n", p=P)[
                        :, :, 512 * u : 512 * (u + 1)
                    ],
                )

            # first attention pair's inputs land first
            dma_wqk(0)
            dma_xt(0)
            dma_wqk(4)
            dma_xt(1)
            nc.sync.dma_start(wv_sb[:], wv_d.rearrange("(t p) n -> p t n", p=P))
            for t in (1, 5, 2, 6, 3, 7):
                dma_wqk(t)
            nc.sync.dma_start(wout_sb[:], wout_d.rearrange("(t p) n -> p t n", p=P))
            nc.vector.memset(vaug_sb[:, :, :, HD : HD + 1], 1.0)
            nc.vector.memset(ones_sb[64:65, :], 1.0)

            # ---- phase 1: QKV projections ----
            def qk_tile(t):
                # col-tile t: t<4 -> q head-pair t ; t>=4 -> k head-pair t-4
                for u in range(2):
                    ps = psum.tile([P, 512], f32, tag="qkv", name=f"qkps{t}{u}")
                    for dt in range(NT):
                        nc.tensor.matmul(
                            ps[:],
                            wqk_sb[:, dt, P * t : P * (t + 1)],
                            xt_sb[:, dt, 512 * u : 512 * (u + 1)],
                            start=(dt == 0),
                            stop=(dt == NT - 1),
                        )
                    dest = qt_sb if t < 4 else kt_sb
                    nc.vector.tensor_copy(
                        dest[:, t % 4, 512 * u : 512 * (u + 1)], ps[:]
                    )

            def v_tile(tt):
                ps = psum.tile([P, 512], f32, tag="qkv", name=f"vps{tt}")
                for dt in range(NT):
                    nc.tensor.matmul(
                        ps[:],
                        xt_sb[:, dt, P * tt : P * (tt + 1)],
                        wv_sb[:, dt, :],
                        start=(dt == 0),
                        stop=(dt == NT - 1),
                    )
                nc.vector.tensor_copy(
                    vaug_sb[:, tt, :, 0:HD], ps[:].rearrange("p (h d) -> p h d", d=HD)
                )

            # pair-0 inputs first so attention can start early
            qk_tile(0)
            qk_tile(4)
            for tt in range(NT):
                v_tile(tt)
            for t in (1, 5, 2, 6, 3, 7):
                qk_tile(t)

            # ---- phase 2: attention, head pairs ----
            for pair in range(HPC // 2):
                heads = (2 * pair, 2 * pair + 1)
                ctx_ps = {
                    h: psum.tile([HD + 1, L], f32, tag="ctx", name=f"ctxps{h}")
                    for h in heads
                }
                for j in range(NT):
                    w = _W[j]
                    boff = pair * _PAIR_COLS + _OFF_J[j]
                    bias_t = bias_pool.tile([P, 2 * L], bf16, tag="bias")
                    nc.sync.dma_start(
                        bias_t[:, : 2 * w], bias_d[:, boff : boff + 2 * w]
                    )
                    for h in heads:
                        base = 64 * (h % 2)
                        for a, b in _regions(j):
                            n = b - a
                            ps = psum.tile([P, 512], f32, tag="sc", name=f"sc{h}{j}{a}")
                            nc.tensor.matmul(
                                ps[:, :n],
                                kt_sb[base : base + 64, pair, P * j : P * (j + 1)],
                                qt_sb[base : base + 64, pair, a:b],
                                start=True,
                                stop=True,
                            )
                            ptr = ptr_pool.tile([P, 512], bf16, tag="ptr")
                            nc.scalar.activation(ptr[:, :n], ps[:, :n], Exp)
                            pt = pt_pool.tile([P, 512], bf16, tag="pt")
                            hw = (h % 2) * w
                            nc.vector.tensor_mul(
                                pt[:, :n],
                                ptr[:, :n],
                                bias_t[:, hw + a - P * j : hw + b - P * j],
                            )
                            if a < 512:
                                st, sp = (j == 0), (j == 3)
                            else:
                                st, sp = (j == 0), (j == NT - 1)
                            nc.tensor.matmul(
                                ctx_ps[h][:, a:b],
                                vaug_sb[:, j, h, :],
                                pt[:, :n],
                                start=st,
                                stop=sp,
                            )
                # normalize by the accumulated denominators (ctx row 64);
                # broadcast the reciprocal row across partitions with a K=1
                # matmul: rbc[0:64, q] = ones[1,64]^T @ recip[1, q]
                for h in heads:
                    cp = ctx_ps[h]
                    rt = r_pool.tile([65, L], bf16, tag="recip")
                    with nc.allow_low_precision(reason="bf16 recip feeds bf16 broadcast matmul"):
                        nc.vector.reciprocal(rt[64:65, :], cp[64:65, :])
                    rbc_sb = r_pool.tile([64, L], f32, tag="rbcsb")
                    for u in range(2):
                        rb = psum.tile([P, 512], f32, tag="qkv", name=f"rbc{h}{u}")
                        nc.tensor.matmul(
                            rb[0:64, :],
                            ones_sb[64:65, :],
                            rt[64:65, 512 * u : 512 * (u + 1)],
                            start=True,
                            stop=True,
                        )
                        nc.vector.tensor_copy(
                            rbc_sb[:, 512 * u : 512 * (u + 1)], rb[0:64, :]
                        )
                    if h % 2 == 0:
                        nc.vector.tensor_mul(
                            ctx_sb[0:64, pair, :], cp[0:64, :], rbc_sb[:]
                        )
                    else:
                        stg = stg_pool.tile([64, L], bf16, tag="stg")
                        nc.vector.tensor_mul(stg[:], cp[0:64, :], rbc_sb[:])
                        nc.sync.dma_start(ctx_sb[64:128, pair, :], stg[:])

            # ---- phase 3: partial output projection ----
            nd = HPC * HD // P  # 4 d-tiles
            for et in range(NT):
                ot = out_pool.tile([P, L], f32, tag="osb")
                for u in range(2):
                    ps = psum.tile([P, 512], f32, tag="qkv", name=f"ops{et}{u}")
                    for dt in range(nd):
                        nc.tensor.matmul(
                            ps[:],
                            wout_sb[:, dt, P * et : P * (et + 1)],
                            ctx_sb[:, dt, 512 * u : 512 * (u + 1)],
                            start=(dt == 0),
                            stop=(dt == nd - 1),
                        )
                    nc.scalar.copy(ot[:, 512 * u : 512 * (u + 1)], ps[:])
                nc.sync.dma_start(outp_d[P * et : P * (et + 1), :], ot[:])

    nc.compile()
    return nc


def _prep_core_inputs(x, mask, w_qkv, w_out, rel_pos_bias):
    """Host-side sharding/layout prep.  Returns in_maps for the 8 cores."""
    w3 = w_qkv.reshape(D, 3, H, HD)
    madd_t = np.where(mask[0, 0], np.float32(0), np.float32(NEG)).T  # [k, q]
    scale = np.float32(HD**-0.5)

    in_maps = []
    for c in range(8):
        b, g = divmod(c, 2)
        hs = slice(g * HPC, (g + 1) * HPC)
        xt = np.ascontiguousarray(x[b].T).astype(BF16)
        qpart = (w3[:, 0, hs, :] * scale).reshape(D, HPC * HD)
        kpart = w3[:, 1, hs, :].reshape(D, HPC * HD)
        wqk_flat = np.concatenate([qpart, kpart], axis=1)  # [D, 1024]
        # pack as 8 stacked col-tile blocks of [D, 128]
        wqk = np.ascontiguousarray(
            wqk_flat.reshape(D, NT, P).transpose(1, 0, 2).reshape(NT * D, P)
        ).astype(BF16)
        wv = np.ascontiguousarray(w3[:, 2, hs, :].reshape(D, HPC * HD)).astype(BF16)
        wout = w_out[g * HPC * HD : (g + 1) * HPC * HD, :].astype(BF16)

        # multiplicative bias table: exp(bias + additive mask), (pair, j)-major
        biasp = np.empty((P, _BIAS_COLS), dtype=np.float32)
        bt = rel_pos_bias[hs].transpose(0, 2, 1)  # [8, k, q]
        for pr in range(HPC // 2):
            for j in range(NT):
                blk = bt[2 * pr : 2 * pr + 2, P * j : P * (j + 1), P * j : L] + madd_t[
                    None, P * j : P * (j + 1), P * j : L
                ]  # [2, 128, W_j]
                o = pr * _PAIR_COLS + _OFF_J[j]
                biasp[:, o : o + 2 * _W[j]] = np.exp(blk).transpose(1, 0, 2).reshape(
                    P, 2 * _W[j]
                )
        in_maps.append(
            {
                "xt": xt,
                "wqk": wqk,
                "wv": wv,
                "wout": wout,
                "biasp": biasp.astype(BF16),
            }
        )
    return in_maps


# test-harness hooks (ignored in normal grading use)
PROFILE_DIR = None
TRACE_CORES = None
LAST_RESULT = None


def kernel(x, mask, w_qkv, w_out, rel_pos_bias):
    from concourse.bass_utils import run_bass_kernel_spmd

    global LAST_RESULT
    nc = _build()
    in_maps = _prep_core_inputs(x, mask, w_qkv, w_out, rel_pos_bias)
    kwargs = {}
    if PROFILE_DIR is not None:
        kwargs = dict(
            trace=True,
            tmpdir=PROFILE_DIR,
            trace_cores=TRACE_CORES,
        )
    res = run_bass_kernel_spmd(nc, in_maps, core_ids=list(range(8)), **kwargs)
    LAST_RESULT = res
    out = np.empty((B, L, D), np.float32)
    for b in range(B):
        part = res.results[2 * b]["outp"] + res.results[2 * b + 1]["outp"]
        out[b] = part.T
    return out


# revision 18
# speedup vs baseline: 1.1412x; 1.1412x over previous
"""Causal multi-head attention (B=4, L=1024, D=1024, H=16) on 8 TRN2 NeuronCores.

Sharding: core c = 2*b + g handles batch b (0..3) and head group g (0..1,
8 heads each).  Each core computes QKV projections for its heads, causal
attention (upper-triangle blocks skipped; the mask and rel-pos bias are
folded into a host-packed MULTIPLICATIVE table exp(bias + mask)), and a
PARTIAL output projection against its 512 rows of w_out.  The two cores of
a batch return partial [D, L] outputs that the host sums and transposes —
no on-device collectives.

Layouts are chosen so nothing is ever transposed on device:
 - qT/kT live as [head_dim(64) on partitions, tok]; the scores matmul
   emits scores^T [kpos, q] directly.
 - v lives as [tok on partitions, 64] with a ones column appended, so the
   ctx matmul ctxT[d, q] = sum_k v[k, d] p[k, q] also accumulates softmax
   denominators into ctx row 64 for free.
 - softmax skips max-subtraction (scores are O(6); exp is safe):
   p = exp(s) * expbias, denominators divide the small [64, L] ctx rows.
 - the denominator reciprocal row is broadcast across partitions with a
   K=1 matmul (ones[1,64]^T @ recip[1,L]) — partition-crossing data moves
   only ever happen on PE or DMA.
"""

import functools

import ml_dtypes
import numpy as np

B, L, D, H = 4, 1024, 1024, 16
HD = D // H  # 64
HPC = H // 2  # heads per core group = 8
P = 128
NT = L // P  # 8 token/query/key tiles
NEG = -1e30

BF16 = ml_dtypes.bfloat16

# packed bias geometry, (pair, j)-major: block (pair, j) holds the head
# pair's two [128, W_j] slabs side by side
_W = [L - P * j for j in range(NT)]
_OFF_J = [0] * NT
for _j in range(1, NT):
    _OFF_J[_j] = _OFF_J[_j - 1] + 2 * _W[_j - 1]
_PAIR_COLS = _OFF_J[-1] + 2 * _W[-1]  # 2 * 4608
_BIAS_COLS = (HPC // 2) * _PAIR_COLS


def _regions(j):
    """q-ranges of the causal suffix [128j, L), split at the 512 psum-bank
    boundary so each matmul output stays inside one bank."""
    a = P * j
    if a < 512:
        return [(a, 512), (512, 1024)]
    return [(a, 1024)]


@functools.lru_cache(maxsize=1)
def _build():
    import concourse.mybir as mybir
    import concourse.tile as tile
    from concourse import bacc

    f32 = mybir.dt.float32
    f32r = mybir.dt.float32r
    bf16 = mybir.dt.bfloat16
    Exp = mybir.ActivationFunctionType.Exp

    nc = bacc.Bacc(None, target_bir_lowering=False, debug=False)

    xt_d = nc.dram_tensor("xt", [D, L], bf16, kind="ExternalInput")
    # wqk packed host-side as 8 col-tile blocks of [D, 128] so each block is
    # one small DMA and the first matmuls start early
    wqk_d = nc.dram_tensor("wqk", [NT * D, P], bf16, kind="ExternalInput")
    wv_d = nc.dram_tensor("wv", [D, HPC * HD], bf16, kind="ExternalInput")
    wout_d = nc.dram_tensor("wout", [HPC * HD, D], bf16, kind="ExternalInput")
    bias_d = nc.dram_tensor("biasp", [P, _BIAS_COLS], bf16, kind="ExternalInput")
    outp_d = nc.dram_tensor("outp", [D, L], f32, kind="ExternalOutput")

    with tile.TileContext(nc) as tc:
        with (
            tc.tile_pool(name="persist", bufs=1) as keep,
            tc.tile_pool(name="bias", bufs=2) as bias_pool,
            tc.tile_pool(name="ptr", bufs=3) as ptr_pool,
            tc.tile_pool(name="pt", bufs=3) as pt_pool,
            tc.tile_pool(name="recip", bufs=2) as r_pool,
            tc.tile_pool(name="stg", bufs=2) as stg_pool,
            tc.tile_pool(name="osb", bufs=2) as out_pool,
            tc.tile_pool(name="psum", bufs=2, space="PSUM") as psum,
        ):
            xt_sb = keep.tile([P, NT, L], bf16, tag="xt")
            wqk_sb = keep.tile([P, NT, 2 * HPC * HD], bf16, tag="wqk")
            wv_sb = keep.tile([P, NT, HPC * HD], bf16, tag="wv")
            wout_sb = keep.tile([P, HPC * HD // P, D], bf16, tag="wout")
            qt_sb = keep.tile([P, HPC // 2, L], bf16, tag="qt")
            kt_sb = keep.tile([P, HPC // 2, L], bf16, tag="kt")
            vaug_sb = keep.tile([P, NT, HPC, HD + 1], bf16, tag="vaug")
            ctx_sb = keep.tile([P, HPC * HD // P, L], bf16, tag="ctx")
            ones_sb = keep.tile([65, HD], bf16, tag="ones")

            def dma_wqk(t):
                nc.sync.dma_start(
                    wqk_sb[:, :, P * t : P * (t + 1)],
                    wqk_d[t * D : (t + 1) * D, :].rearrange("(dt p) c -> p dt c", p=P),
                )

            def dma_xt(u):
                nc.sync.dma_start(
                    xt_sb[:, :, 512 * u : 512 * (u + 1)],
                    xt_d.rearrange("(t p) n -> p t n", p=P)[
                        :, :, 512 * u : 512 * (u + 1)
                    ],
                )

            # first attention pair's inputs land first
            dma_wqk(0)
            dma_xt(0)
            dma_wqk(4)
            dma_xt(1)
            nc.sync.dma_start(wv_sb[:], wv_d.rearrange("(t p) n -> p t n", p=P))
            for t in (1, 5, 2, 6, 3, 7):
                dma_wqk(t)
            nc.sync.dma_start(wout_sb[:], wout_d.rearrange("(t p) n -> p t n", p=P))
            nc.vector.memset(vaug_sb[:, :, :, HD : HD + 1], 1.0)
            nc.vector.memset(ones_sb[64:65, :], 1.0)

            # ---- phase 1: QKV projections ----
            def qk_tile(t):
                # col-tile t: t<4 -> q head-pair t ; t>=4 -> k head-pair t-4
                for u in range(2):
                    ps = psum.tile([P, 512], f32, tag="qkv", name=f"qkps{t}{u}")
                    for dt in range(NT):
                        nc.tensor.matmul(
                            ps[:],
                            wqk_sb[:, dt, P * t : P * (t + 1)],
                            xt_sb[:, dt, 512 * u : 512 * (u + 1)],
                            start=(dt == 0),
                            stop=(dt == NT - 1),
                        )
                    dest = qt_sb if t < 4 else kt_sb
                    nc.vector.tensor_copy(
                        dest[:, t % 4, 512 * u : 512 * (u + 1)], ps[:]
                    )

            def v_tile(tt):
                ps = psum.tile([P, 512], f32, tag="qkv", name=f"vps{tt}")
                for dt in range(NT):
                    nc.tensor.matmul(
                        ps[:],
                        xt_sb[:, dt, P * tt : P * (tt + 1)],
                        wv_sb[:, dt, :],
                        start=(dt == 0),
                        stop=(dt == NT - 1),
                    )
                nc.vector.tensor_copy(
                    vaug_sb[:, tt, :, 0:HD], ps[:].rearrange("p (h d) -> p h d", d=HD)
                )

            # pair-0 inputs first so attention can start early
            qk_tile(0)
            qk_tile(4)
            for tt in range(NT):
                v_tile(tt)
            for t in (1, 5, 2, 6, 3, 7):
                qk_tile(t)

            # ---- phase 2: attention, head pairs ----
            for pair in range(HPC // 2):
                heads = (2 * pair, 2 * pair + 1)
                ctx_ps = {
                    h: psum.tile([HD + 1, L], f32, tag="ctx", name=f"ctxps{h}")
                    for h in heads
                }
                for j in range(NT):
                    w = _W[j]
                    boff = pair * _PAIR_COLS + _OFF_J[j]
                    bias_t = bias_pool.tile([P, 2 * L], bf16, tag="bias")
                    nc.sync.dma_start(
                        bias_t[:, : 2 * w], bias_d[:, boff : boff + 2 * w]
                    )
                    for h in heads:
                        base = 64 * (h % 2)
                        for a, b in _regions(j):
                            n = b - a
                            ps = psum.tile([P, 512], f32, tag="sc", name=f"sc{h}{j}{a}")
                            nc.tensor.matmul(
                                ps[:, :n],
                                kt_sb[base : base + 64, pair, P * j : P * (j + 1)],
                                qt_sb[base : base + 64, pair, a:b],
                                start=True,
                                stop=True,
                            )
                            ptr = ptr_pool.tile([P, 512], bf16, tag="ptr")
                            nc.scalar.activation(ptr[:, :n], ps[:, :n], Exp)
                            pt = pt_pool.tile([P, 512], bf16, tag="pt")
                            hw = (h % 2) * w
                            nc.vector.tensor_mul(
                                pt[:, :n],
                                ptr[:, :n],
                                bias_t[:, hw + a - P * j : hw + b - P * j],
                            )
                            if a < 512:
                                st, sp = (j == 0), (j == 3)
                            else:
                                st, sp = (j == 0), (j == NT - 1)
                            nc.tensor.matmul(
                                ctx_ps[h][:, a:b],
                                vaug_sb[:, j, h, :],
                                pt[:, :n],
                                start=st,
                                stop=sp,
                            )
                # normalize by the accumulated denominators (ctx row 64).
                # One fast DVE copy evacuates raw ctx+sums to SBUF so the
                # psum slot frees immediately and PE never stalls on this
                # chain.  1/s is exp(-ln(s)) on ACT: the DVE reciprocal
                # measures ~6.5us for a 1024-wide row; ACT ln+exp is ~2us
                # and shares the natural_log_exp table set with the hot exp.
                for h in heads:
                    cp = ctx_ps[h]
                    craw = r_pool.tile([65, L], f32, tag="craw")
                    nc.vector.tensor_copy(craw[:], cp[:])
                    lns = r_pool.tile([65, L], f32, tag="lns")
                    nc.scalar.activation(
                        lns[64:65, :], craw[64:65, :],
                        mybir.ActivationFunctionType.Ln,
                    )
                    rt = r_pool.tile([65, L], bf16, tag="recip")
                    with nc.allow_low_precision(reason="bf16 recip, bf16 bcast mm"):
                        nc.scalar.activation(rt[64:65, :], lns[64:65, :], Exp,
                                             scale=-1.0)
                    rbc = {}
                    for u in range(2):
                        rb = psum.tile([P, 512], f32, tag="qkv", name=f"rbc{h}{u}")
                        nc.tensor.matmul(
                            rb[0:64, :],
                            ones_sb[64:65, :],
                            rt[64:65, 512 * u : 512 * (u + 1)],
                            start=True,
                            stop=True,
                        )
                        rbc[u] = rb
                    if h % 2 == 0:
                        dst = ctx_sb[0:64, pair, :]
                    else:
                        stg = stg_pool.tile([64, L], bf16, tag="stg")
                        dst = stg[:]
                    for u in range(2):
                        nc.vector.tensor_mul(
                            dst[:, 512 * u : 512 * (u + 1)],
                            craw[0:64, 512 * u : 512 * (u + 1)],
                            rbc[u][0:64, :],
                        )
                    if h % 2 == 1:
                        nc.sync.dma_start(ctx_sb[64:128, pair, :], dst)

            # ---- phase 3: partial output projection ----
            nd = HPC * HD // P  # 4 d-tiles
            for et in range(NT):
                ot = out_pool.tile([P, L], f32, tag="osb")
                for u in range(2):
                    ps = psum.tile([P, 512], f32, tag="qkv", name=f"ops{et}{u}")
                    for dt in range(nd):
                        nc.tensor.matmul(
                            ps[:],
                            wout_sb[:, dt, P * et : P * (et + 1)],
                            ctx_sb[:, dt, 512 * u : 512 * (u + 1)],
                            start=(dt == 0),
                            stop=(dt == nd - 1),
                        )
                    nc.scalar.copy(ot[:, 512 * u : 512 * (u + 1)], ps[:])
                nc.sync.dma_start(outp_d[P * et : P * (et + 1), :], ot[:])

    nc.compile()
    return nc


def _prep_core_inputs(x, mask, w_qkv, w_out, rel_pos_bias):
    """Host-side sharding/layout prep.  Returns in_maps for the 8 cores."""
    w3 = w_qkv.reshape(D, 3, H, HD)
    madd_t = np.where(mask[0, 0], np.float32(0), np.float32(NEG)).T  # [k, q]
    scale = np.float32(HD**-0.5)

    in_maps = []
    for c in range(8):
        b, g = divmod(c, 2)
        hs = slice(g * HPC, (g + 1) * HPC)
        xt = np.ascontiguousarray(x[b].T).astype(BF16)
        qpart = (w3[:, 0, hs, :] * scale).reshape(D, HPC * HD)
        kpart = w3[:, 1, hs, :].reshape(D, HPC * HD)
        wqk_flat = np.concatenate([qpart, kpart], axis=1)  # [D, 1024]
        # pack as 8 stacked col-tile blocks of [D, 128]
        wqk = np.ascontiguousarray(
            wqk_flat.reshape(D, NT, P).transpose(1, 0, 2).reshape(NT * D, P)
        ).astype(BF16)
        wv = np.ascontiguousarray(w3[:, 2, hs, :].reshape(D, HPC * HD)).astype(BF16)
        wout = w_out[g * HPC * HD : (g + 1) * HPC * HD, :].astype(BF16)

        # multiplicative bias table: exp(bias + additive mask), (pair, j)-major
        biasp = np.empty((P, _BIAS_COLS), dtype=np.float32)
        bt = rel_pos_bias[hs].transpose(0, 2, 1)  # [8, k, q]
        for pr in range(HPC // 2):
            for j in range(NT):
                blk = bt[2 * pr : 2 * pr + 2, P * j : P * (j + 1), P * j : L] + madd_t[
                    None, P * j : P * (j + 1), P * j : L
                ]  # [2, 128, W_j]
                o = pr * _PAIR_COLS + _OFF_J[j]
                biasp[:, o : o + 2 * _W[j]] = np.exp(blk).transpose(1, 0, 2).reshape(
                    P, 2 * _W[j]
                )
        in_maps.append(
            {
                "xt": xt,
                "wqk": wqk,
                "wv": wv,
                "wout": wout,
                "biasp": biasp.astype(BF16),
            }
        )
    return in_maps


# test-harness hooks (ignored in normal grading use)
PROFILE_DIR = None
TRACE_CORES = None
LAST_RESULT = None


def kernel(x, mask, w_qkv, w_out, rel_pos_bias):
    from concourse.bass_utils import run_bass_kernel_spmd

    global LAST_RESULT
    nc = _build()
    in_maps = _prep_core_inputs(x, mask, w_qkv, w_out, rel_pos_bias)
    kwargs = {}
    if PROFILE_DIR is not None:
        kwargs = dict(
            trace=True,
            tmpdir=PROFILE_DIR,
            trace_cores=TRACE_CORES,
        )
    res = run_bass_kernel_spmd(nc, in_maps, core_ids=list(range(8)), **kwargs)
    LAST_RESULT = res
    out = np.empty((B, L, D), np.float32)
    for b in range(B):
        part = res.results[2 * b]["outp"] + res.results[2 * b + 1]["outp"]
        out[b] = part.T
    return out
